# revision 32
# baseline (speedup 1.0000x reference)
"""Multi-head attention (B=8, N=1024, C=768, 12 heads) on 8 TRN2 NeuronCores.

Sharding: data-parallel over batch — batch element b runs on core b, weights
replicated, zero collectives.

Per-core kernel (all matmuls bf16 on the TensorEngine). Structure relative
to the measured HW model:
  - per-MM floor ~256ns (N=512 stream 213ns + ~43ns issue/sem overhead);
  - a 64-row-tiled K=64 score pair runs CONCURRENTLY in ~295ns (measured),
    but every (64,128)<->(128,128) tiling-mode change costs a ~113ns drain;
  - ScalarE exp is (N+352)/1.2 ns -> ~1300ns per [128,1024] step, 125us
    total for 96 steps — slightly below the PE's total work, so the loop
    is PE-paced and every PE cycle saved is wall time.

Design:
  - scores are computed TRANSPOSED, S^T[k,q], lhsT=k^T, rhs=q^T, as 12
    passes (head pair p=i//2, q-half qh=i%2) of 4 BLOCKS of 2 kc-steps.
    Each block issues its 2 steps' score pairs back-to-back in 64-row
    tiled mode (4 MMs, 2 mode switches per block instead of 4), then an
    untiled segment: P@V of the PREVIOUS pass (2 steps' worth, evenly
    spread), plus one or two filler chains.
  - exp (ScalarE) is issued right after each score pair; softmax
    denominators come free via a ones-column in v (row 64 of the P@V
    accumulator).
  - qkv projection runs as filler chains dispensed into the untiled
    segments, deadline-ordered; only q/k of head pair 0 is computed in
    the prologue (interleaved with the x-chunk DMAs, chunk by chunk).
    DMA order: Wqkv[q0k0] -> x -> Wqkv[v] -> Wqkv[q1k1] -> rest -> Wproj,
    so no chain ever waits on a late weight block.
  - projection y = attn @ W_proj^T + b_proj is split per 128-row chunk
    into a lo [*,512] and hi [*,256] region, each accumulated in TWO
    PSUM sessions: A = c0..2 (dispensed into passes 7-9), B = c3..5
    (pass 11 / epilogue); bias is added by DVE during A's PSUM->SBUF
    staging and the final y = A_sb + B_ps add lands in the output stage
    tile. No tile-pool release barriers anywhere.
"""

from collections import deque
from contextlib import ExitStack

import ml_dtypes
import numpy as np

import concourse.mybir as mybir
import concourse.tile as tile
from concourse import bacc
from concourse.bass_utils import run_bass_kernel_spmd

B, N, C = 8, 1024, 768
NH, D = 12, 64
CK = C // 128  # 6 contraction chunks of 128
NQ = N // 128  # 8 position chunks of 128
SCALE = D ** -0.5
F32 = mybir.dt.float32
BF16 = mybir.dt.bfloat16
Copy = mybir.ActivationFunctionType.Copy
Exp = mybir.ActivationFunctionType.Exp
BF = ml_dtypes.bfloat16


def _emit(tc, xT, wqkvT, wprojT, bproj, out, sim_safe=False):
    nc = tc.nc
    with ExitStack() as ctx:
        sb = ctx.enter_context(tc.tile_pool(name="sb", bufs=1))
        pp = ctx.enter_context(tc.tile_pool(name="pp", bufs=13))
        small = ctx.enter_context(tc.tile_pool(name="small", bufs=2))
        stage = ctx.enter_context(tc.tile_pool(name="stage", bufs=2))
        # PSUM budget (8 banks): score ring 2x[128,1024] (4 banks) + P@V
        # accumulator ring 3x[65,512] (3 banks) + filler/proj chain (1).
        acc = tc.alloc_tile_pool(name="acc", bufs=3, space="PSUM")
        ps = tc.alloc_tile_pool(name="ps", bufs=2, space="PSUM")
        fill = tc.alloc_tile_pool(name="fill", bufs=1, space="PSUM")
        warm_ps = ps.tile([128, 512], F32, name="warm_ps", tag="s")

        # ---- PE warm-up ----------------------------------------------
        # ~3us of matmuls so the HAM clock-gate opens (K=8/8) before the
        # first qk chains; the chains themselves keep it open.
        # ---- input DMAs (bf16, host-packed) --------------------------
        xT_bf = [
            sb.tile([128, N], BF16, name=f"xT_bf{c}", tag=f"xT_bf{c}")
            for c in range(CK)
        ]
        wq_bf = sb.tile([128, CK, 3 * C], BF16, name="wq_bf", tag="wq_bf")
        wp_bf = sb.tile([128, CK, C], BF16, name="wp_bf", tag="wp_bf")

        # warm-up matmuls read the NOT-YET-DMA'd W_proj region: garbage
        # values (results are discarded; the psum slot is overwritten by
        # the first score matmul's start=True), but zero gating — unlike
        # a memset, whose engine queue only starts ~4us in. The implied
        # read-before-write ordering delays only the W_proj DMA, which is
        # last in the queue and not needed until pass 7. CoreSim rejects
        # uninitialized reads, so sim_safe swaps in a memset-fed warmup
        # (numerically identical: the output is discarded either way).
        if sim_safe:
            warm_in = sb.tile([128, 512], BF16, name="warm_in", tag="warm_in")
            nc.gpsimd.memset(warm_in[:], 1.0)
            w_lhs, w_rhs = warm_in[:, 0:128], warm_in[:]
        else:
            w_lhs, w_rhs = wp_bf[:, 0, 0:128], wp_bf[:, 0, 0:512]
        # 11 matmuls ~= 4.7us at the cold clock: covers the HAM SHORT
        # window AND bridges to ~12us, where the first chain matmul can
        # actually issue (queue startup + DMA) — any PE idle gap here
        # would re-throttle the clock for the first chain MMs.
        for i in range(11):
            nc.tensor.matmul(
                warm_ps[:],
                lhsT=w_lhs,
                rhs=w_rhs,
                start=(i == 0),
                stop=(i == 10),
            )

        def dma_w(lo, hi):
            nc.sync.dma_start(
                out=wq_bf[:, :, lo:hi],
                in_=wqkvT[:, lo:hi].rearrange("(c p) w -> p c w", p=128),
            )

        dma_w(0, 256)  # q0|k0 -> prologue chains
        for c in range(CK):
            nc.sync.dma_start(
                out=xT_bf[c][:], in_=xT[c * 128:(c + 1) * 128, :]
            )
        dma_w(1536, 2304)  # v weights -> v filler chains (pass 0)
        dma_w(256, 512)  # q1|k1 -> qk(1)/qk(7) fillers (passes 0-1)
        dma_w(512, 1536)  # remaining q/k blocks
        nc.sync.dma_start(
            out=wp_bf[:], in_=wprojT.rearrange("(c p) w -> p c w", p=128)
        )
        bp_row = sb.tile([1, C], F32, name="bp_row", tag="bp_row")
        nc.sync.dma_start(out=bp_row[:], in_=bproj[None, :])

        # ---- qkv projections (filler chains) -------------------------
        # qkT[m] m 0..5 -> q rows of heads 2m,2m+1; 6..11 -> k rows.
        # Packed weight col offset: q_m at 256m, k_m at 256m+128.
        qkT = [
            sb.tile([128, N], BF16, name=f"qkT{m}", tag=f"qkT{m}")
            for m in range(12)
        ]

        def emit_qk_half(m, qh, pool=None, ptag="f"):
            co = 256 * m if m < 6 else 256 * (m - 6) + 128
            pool = pool or fill
            qk_ps = pool.tile([128, 512], F32, name=f"qk_ps{m}_{qh}", tag=ptag)
            for c in range(CK):
                nc.tensor.matmul(
                    qk_ps[:],
                    lhsT=wq_bf[:, c, co:co + 128],
                    rhs=xT_bf[c][:, qh * 512:(qh + 1) * 512],
                    start=(c == 0),
                    stop=(c == CK - 1),
                )
            nc.vector.tensor_copy(qkT[m][:, qh * 512:(qh + 1) * 512], qk_ps[:])

        v_sb = [
            sb.tile([128, NH, D + 1], BF16, name=f"v_sb{n}", tag=f"v_sb{n}")
            for n in range(NQ)
        ]

        def emit_v_half(n, half, pool=None, ptag="f"):
            if half == 0:
                nc.gpsimd.memset(v_sb[n][:, :, D], 1.0)
            pool = pool or fill
            v_ps = pool.tile([128, 384], F32, name=f"v_ps{n}_{half}", tag=ptag)
            for c in range(CK):
                nc.tensor.matmul(
                    v_ps[:],
                    lhsT=xT_bf[c][:, n * 128:(n + 1) * 128],
                    rhs=wq_bf[:, c, 1536 + half * 384:1536 + (half + 1) * 384],
                    start=(c == 0),
                    stop=(c == CK - 1),
                )
            nc.vector.tensor_copy(
                v_sb[n][:, half * 6:(half + 1) * 6, 0:D],
                v_ps[:].rearrange("p (h d) -> p h d", d=D),
            )

        # ---- prologue: q/k of head pair 0, DMA-pipelined -------------
        # The two chains interleave by c-chunk: 2 MMs per x chunk keeps
        # each per-chunk DMA wait short (~0.2us — far below the ~3.4us
        # idle window that would re-throttle the HAM clock gate, which a
        # sequential chain's ~1.5us waits do trip). Copies follow both.
        pA = ps.tile([128, 512], F32, name="pre0_0", tag="s")
        pB = ps.tile([128, 512], F32, name="pre6_0", tag="s")
        for c in range(CK):
            nc.tensor.matmul(
                pA[:], lhsT=wq_bf[:, c, 0:128], rhs=xT_bf[c][:, 0:512],
                start=(c == 0), stop=(c == CK - 1),
            )
            nc.tensor.matmul(
                pB[:], lhsT=wq_bf[:, c, 128:256], rhs=xT_bf[c][:, 0:512],
                start=(c == 0), stop=(c == CK - 1),
            )
        nc.vector.tensor_copy(qkT[0][:, 0:512], pA[:])
        nc.vector.tensor_copy(qkT[6][:, 0:512], pB[:])

        # ---- attention: 12 passes x 4 blocks of 2 steps --------------
        attn_bf = [
            sb.tile([128, N], BF16, name=f"attn_bf{p}", tag=f"attn_bf{p}")
            for p in range(6)
        ]

        def emit_S(i, kc):
            """Row-tiled score pair + exp for pass i=(p,qh), chunk kc."""
            p, qh = i // 2, i % 2
            q_tile, k_tile = qkT[p], qkT[6 + p]
            qs = slice(qh * 512, (qh + 1) * 512)
            st = ps.tile([128, N], F32, name=f"st{i}_{kc}", tag="s")
            nc.tensor.matmul(
                st[:, 0:512],
                lhsT=k_tile[0:D, kc * 128:(kc + 1) * 128],
                rhs=q_tile[0:D, qs],
                start=True,
                stop=True,
            )
            nc.tensor.matmul(
                st[:, 512:1024],
                lhsT=k_tile[D:128, kc * 128:(kc + 1) * 128],
                rhs=q_tile[D:128, qs],
                start=True,
                stop=True,
            )
            pt = pp.tile([128, N], BF16, name=f"P{i}_{kc}", tag="P")
            nc.scalar.activation(pt[:], st[:], Exp, scale=SCALE)
            return pt

        def emit_pv(i, oas, kc, pt):
            """P@V chunk kc for both heads of pass i=(p,qh)."""
            p = i // 2
            for half, oa in enumerate(oas):
                nc.tensor.matmul(
                    oa[:],
                    lhsT=v_sb[kc][:, 2 * p + half, :],
                    rhs=pt[:, half * 512:(half + 1) * 512],
                    start=(kc == 0),
                    stop=(kc == NQ - 1),
                )

        def emit_norm_pre(oas):
            """Reciprocal chain for the pass's two heads (DVE/GpSimd).
            Kept per-half: the two ~1us GpSimd broadcasts pipeline with
            other work, which beats one [64,1024] broadcast's latency."""
            bc = []
            for half in range(2):
                dn = small.tile([1, 512], F32, name=f"dn{half}", tag=f"dn{half}")
                nc.vector.tensor_copy(dn[:], oas[half][D:D + 1, :])
                rc = small.tile([1, 512], F32, name=f"rc{half}", tag=f"rc{half}")
                nc.vector.reciprocal_approx_fast(rc[:], dn[:])
                rcb = small.tile([1, 512], BF16, name=f"rcb{half}", tag=f"rcb{half}")
                nc.vector.tensor_copy(rcb[:], rc[:])
                b = small.tile([64, 512], BF16, name=f"bc{half}", tag=f"bc{half}")
                nc.gpsimd.partition_broadcast(b[:], rcb[:])
                bc.append(b)
            return bc

        def emit_norm_post(i, oas, bc):
            p, qh = i // 2, i % 2
            qs = slice(qh * 512, (qh + 1) * 512)
            for half in range(2):
                ro = half * 64
                nc.vector.tensor_mul(
                    attn_bf[p][ro:ro + 64, qs], oas[half][0:D, :], bc[half][:]
                )

        # ---- projection chains (A: c0-2+bias, B1: c3-4, C: c5) -------
        # y[n] = attn[n] @ Wp^T + b accumulates in three PSUM sessions so
        # nearly all of it hides inside the pass loop: A as soon as pairs
        # 0-2 are normalized (passes 8-9), B1 once pairs 3-4 are (passes
        # 10-11), and only the single-MM C closer (pair 5) + final add +
        # DMA remain in the epilogue.
        bias_bc = sb.tile([128, C], F32, name="bias_bc", tag="bias_bc")
        nc.gpsimd.partition_broadcast(bias_bc[:], bp_row[:])
        regions = [(0, 512), (512, 768)]
        projA_sb = [
            sb.tile([128, C], F32, name=f"pA{n}", tag=f"pA{n}") for n in range(NQ)
        ]
        projB_sb = [
            sb.tile([128, C], F32, name=f"pB{n}", tag=f"pB{n}") for n in range(NQ)
        ]

        def emit_projA(n, r):
            lo, hi = regions[r]
            a_ps = fill.tile([128, hi - lo], F32, name=f"aps{n}_{r}", tag="f")
            for c in range(3):
                nc.tensor.matmul(
                    a_ps[:],
                    lhsT=attn_bf[c][:, n * 128:(n + 1) * 128],
                    rhs=wp_bf[:, c, lo:hi],
                    start=(c == 0),
                    stop=(c == 2),
                )
            nc.vector.tensor_add(projA_sb[n][:, lo:hi], a_ps[:], bias_bc[:, lo:hi])

        def emit_projB1(n, r):
            lo, hi = regions[r]
            b_ps = fill.tile([128, hi - lo], F32, name=f"bps{n}_{r}", tag="f")
            for c in (3, 4):
                nc.tensor.matmul(
                    b_ps[:],
                    lhsT=attn_bf[c][:, n * 128:(n + 1) * 128],
                    rhs=wp_bf[:, c, lo:hi],
                    start=(c == 3),
                    stop=(c == 4),
                )
            nc.vector.tensor_add(
                projB_sb[n][:, lo:hi], b_ps[:], projA_sb[n][:, lo:hi]
            )

        def emit_projC_region(n, r):
            """In-pass closer (fill bank): c5 for one region + add."""
            lo, hi = regions[r]
            c_ps = fill.tile([128, hi - lo], F32, name=f"cps{n}_{r}", tag="f")
            nc.tensor.matmul(
                c_ps[:],
                lhsT=attn_bf[5][:, n * 128:(n + 1) * 128],
                rhs=wp_bf[:, 5, lo:hi],
                start=True,
                stop=True,
            )
            if r == 0:
                y_stage[n] = stage.tile([128, C], F32, name=f"y{n}", tag="y")
            nc.vector.tensor_add(y_stage[n][:, lo:hi], c_ps[:], projB_sb[n][:, lo:hi])
            if r == 1:
                nc.sync.dma_start(
                    out=out[n * 128:(n + 1) * 128, :], in_=y_stage[n][:]
                )

        def emit_projB_open(n, c, c_ps=None):
            """Epilogue n>=4 session (paused): one c's region MMs into a
            2-bank ps-pool tile; runs inside the PV(11)/norm window."""
            if c_ps is None:
                c_ps = ps.tile([128, C], F32, name=f"cps{n}", tag="s")
            for lo, hi in regions:
                nc.tensor.matmul(
                    c_ps[:, lo:hi],
                    lhsT=attn_bf[c][:, n * 128:(n + 1) * 128],
                    rhs=wp_bf[:, c, lo:hi],
                    start=(c == 3),
                    stop=False,
                )
            return c_ps

        def emit_projB_close(n, c_ps):
            """c5 region MMs (valid after norm(11)), ONE [128,768] add
            against the session-A partial, DMA."""
            for lo, hi in regions:
                nc.tensor.matmul(
                    c_ps[:, lo:hi],
                    lhsT=attn_bf[5][:, n * 128:(n + 1) * 128],
                    rhs=wp_bf[:, 5, lo:hi],
                    start=False,
                    stop=True,
                )
            y_stage[n] = stage.tile([128, C], F32, name=f"y{n}", tag="y")
            nc.vector.tensor_add(y_stage[n][:], c_ps[:], projA_sb[n][:])
            nc.sync.dma_start(out=out[n * 128:(n + 1) * 128, :], in_=y_stage[n][:])

        y_stage = [None] * NQ

        # ---- fill queues ---------------------------------------------
        # fills: deadline-ordered qkv chains. v(n,0) consumed by P@V of
        # pass 0 (during pass 1, block n//2); qk(m)/qk(6+m) by pass 2m;
        # v(n,1) first consumed by P@V of pass 6 (during pass 7).
        fills = deque()
        fills.append(lambda: emit_qk_half(6, 1))  # k pair0 pos 512:1024, pass 0 block 2
        fills.append(lambda: emit_qk_half(0, 1))  # q pair0 half 1, pass 1
        for n in range(NQ):
            fills.append(lambda n=n: emit_v_half(n, 0))
        for m in (7, 1):
            fills.append(lambda m=m: emit_qk_half(m, 0))
        for m in (7, 1):
            fills.append(lambda m=m: emit_qk_half(m, 1))
        for m in (8, 2):
            fills.append(lambda m=m: emit_qk_half(m, 0))
        for m in (8, 2):
            fills.append(lambda m=m: emit_qk_half(m, 1))
        for m in (9, 3):
            fills.append(lambda m=m: emit_qk_half(m, 0))
        for m in (9, 3):
            fills.append(lambda m=m: emit_qk_half(m, 1))
        for n in range(NQ):
            fills.append(lambda n=n: emit_v_half(n, 1))
        for m in (10, 4):
            fills.append(lambda m=m: emit_qk_half(m, 0))
        for m in (10, 4):
            fills.append(lambda m=m: emit_qk_half(m, 1))
        for m in (11, 5):
            fills.append(lambda m=m: emit_qk_half(m, 0))
        for m in (11, 5):
            fills.append(lambda m=m: emit_qk_half(m, 1))

        projA_q = deque((n, r) for n in range(NQ) for r in range(2))

        NPASS = 12
        prev = None  # (i, pts) of previous pass (P@V pending this pass)
        pend_post = None  # (i, oas, bc) awaiting norm_post
        for i in range(NPASS):
            if prev is not None:
                pi, ppts = prev
                poas = (
                    acc.tile([D + 1, 512], F32, name=f"oaA{pi}", tag="acc"),
                    acc.tile([D + 1, 512], F32, name=f"oaB{pi}", tag="acc"),
                )
            pts = {}
            for b in range(4):
                # tiled segment: this block's two score pairs + exps
                pts[2 * b] = emit_S(i, 2 * b)
                pts[2 * b + 1] = emit_S(i, 2 * b + 1)
                # untiled segment. norm_post first (frees the acc slot
                # this pass's P@V reuses), then fillers/proj chains —
                # independent PE work that covers the norm chain's
                # DVE/GpSimd latency — then the P@V matmuls.
                if b == 0 and pend_post is not None:
                    emit_norm_post(*pend_post)
                    pend_post = None
                nfill = 2 if i < 2 else 1
                for _ in range(nfill):
                    if fills:
                        fills.popleft()()
                if not fills and i <= 9 and projA_q:
                    emit_projA(*projA_q.popleft())
                    if projA_q:
                        emit_projA(*projA_q.popleft())
                # B1 sessions: pairs 3-4 rows are normalized by pass 10
                # block 0 (qh1 of pair 4 via norm_post(9) there), so n<4
                # chains run through pass 10 and n>=4 through pass 11.
                if i == 10:
                    emit_projB1(b, 0)
                    emit_projB1(b, 1)
                if prev is not None:
                    # pass 11 front-loads P@V(10) into blocks 0-1 so its
                    # normalize finishes in-pass and the pair-5-qh0 proj
                    # closers can run in blocks 2-3.
                    ks = (
                        (2 * b, 2 * b + 1) if i < 11
                        else (4 * b, 4 * b + 1, 4 * b + 2, 4 * b + 3) if b < 2
                        else ()
                    )
                    for k in ks:
                        emit_pv(pi, poas, k, ppts.pop(k))
                    if i == 11 and b == 1:
                        pend_post = (pi, poas, emit_norm_pre(poas))
                    if i == 11 and b == 2 and pend_post is not None:
                        emit_norm_post(*pend_post)
                        pend_post = None
                        # pass 11's own P@V accumulators: the acc slots
                        # they rotate onto were just freed by the two
                        # norm_posts above (9 at b0, 10 here)
                        loas = (
                            acc.tile([D + 1, 512], F32, name="oaA11", tag="acc"),
                            acc.tile([D + 1, 512], F32, name="oaB11", tag="acc"),
                        )
                # pair-5-qh0 closers once norm_post(10) has landed, and
                # the first half of P@V(11) — its exps are long done
                if i == 11 and b >= 2:
                    for nn in ((0,) if b == 2 else (1, 2)):
                        emit_projC_region(nn, 0)
                        emit_projC_region(nn, 1)
                    for k in (2 * (b - 2), 2 * (b - 2) + 1):
                        emit_pv(i, loas, k, pts.pop(k))
            if i < 11 and prev is not None:
                pend_post = (pi, poas, emit_norm_pre(poas))
            prev = (i, pts)

        # ---- epilogue ------------------------------------------------
        # P@V + normalize of pass 11, C closers (c5) + final adds + DMA.
        li, lpts = prev
        opens = {}
        for kc in range(4, NQ):
            emit_pv(li, loas, kc, lpts.pop(kc))
            # last n<4 closer (norm_post(10) landed in pass 11 block 2)
            if kc == 4:
                emit_projC_region(3, 0)
                opens[4] = emit_projB_open(4, 3)
            elif kc == 5:
                emit_projC_region(3, 1)
                emit_projB_open(4, 4, opens[4])
            elif kc == 6:
                opens[5] = emit_projB_open(5, 3)
            elif kc == 7:
                emit_projB_open(5, 4, opens[5])
        fill.release()
        emit_norm_post(li, loas, emit_norm_pre(loas))
        # pair5 qh1 now normalized: c5 closers + one add + DMA per n
        emit_projB_close(4, opens[4])
        emit_projB_close(5, opens[5])
        for n in (6, 7):
            c_ps = emit_projB_open(n, 3)
            emit_projB_open(n, 4, c_ps)
            emit_projB_close(n, c_ps)
        ps.release()
        acc.release()


def build_graph(sim_safe=False):
    nc = bacc.Bacc("TRN2", target_bir_lowering=False, debug=False)
    xT = nc.declare_dram_parameter("xT", [C, N], BF16, isOutput=False)
    wqkvT = nc.declare_dram_parameter("wqkvT", [C, 3 * C], BF16, isOutput=False)
    wprojT = nc.declare_dram_parameter("wprojT", [C, C], BF16, isOutput=False)
    bproj = nc.declare_dram_parameter("bproj", [C], F32, isOutput=False)
    out = nc.declare_dram_parameter("out", [N, C], F32, isOutput=True)
    with tile.TileContext(nc) as tc:
        _emit(tc, xT.ap(), wqkvT.ap(), wprojT.ap(), bproj.ap(), out.ap(), sim_safe)
    nc.compile()
    return nc


_GRAPH = None


def _get_graph():
    global _GRAPH
    if _GRAPH is None:
        _GRAPH = build_graph()
    return _GRAPH


def make_in_maps(x, W_qkv, W_proj, b_proj):
    x = np.asarray(x, dtype=np.float32)
    wq = np.asarray(W_qkv, dtype=np.float32).T  # [C, 3C]; cols q|k|v
    # pack q/k column blocks interleaved: [q0|k0|q1|k1|...|q5|k5|v]
    packed = np.empty((C, 3 * C), dtype=BF)
    for m in range(6):
        packed[:, 256 * m:256 * m + 128] = wq[:, 128 * m:128 * (m + 1)]
        packed[:, 256 * m + 128:256 * (m + 1)] = wq[:, C + 128 * m:C + 128 * (m + 1)]
    packed[:, 1536:] = wq[:, 1536:]
    wprojT = np.ascontiguousarray(np.asarray(W_proj, dtype=np.float32).T.astype(BF))
    bp = np.ascontiguousarray(np.asarray(b_proj, dtype=np.float32))
    xT_all = np.ascontiguousarray(x.transpose(0, 2, 1).astype(BF))
    return [
        {"xT": xT_all[i], "wqkvT": packed, "wprojT": wprojT, "bproj": bp}
        for i in range(B)
    ]


def run(x, W_qkv, W_proj, b_proj, trace=False):
    nc = _get_graph()
    in_maps = make_in_maps(x, W_qkv, W_proj, b_proj)
    res = run_bass_kernel_spmd(nc, in_maps, core_ids=list(range(B)), trace=trace)
    out = np.stack([res.results[i]["out"] for i in range(B)], axis=0)
    return out.astype(np.float32, copy=False), res


def kernel(x, W_qkv, W_proj, b_proj, H=None, W=None):
    out, _ = run(x, W_qkv, W_proj, b_proj)
    return out


# revision 33
# speedup vs baseline: 1.0010x; 1.0010x over previous
"""Multi-head attention (B=8, N=1024, C=768, 12 heads) on 8 TRN2 NeuronCores.

Sharding: data-parallel over batch - batch element b runs on core b, weights
replicated, zero collectives.

Per-core kernel (all matmuls bf16 on the TensorEngine). Measured HW model
this schedule is built around:
  - back-to-back 512-free matmuls issue every ~216ns (512/2.4GHz + NX);
  - a 64-row-tiled K=64 score pair streams CONCURRENTLY (~295ns for both
    heads), but each (64,128)<->(128,128) tiling-mode change costs a
    ~113ns drain, so score pairs are batched two steps per mode switch;
  - ScalarE exp costs (N+352)/1.2 ns -> ~1150ns per [128,1024] step;
    96 steps = ~110us, slightly below the PE's ~142us of in-loop work,
    so the loop is PE-paced and every PE matmul-cycle saved is wall time;
  - the HAM clock gate holds the PE at 1.2GHz until it has been busy
    ~3.4us, and re-throttles after similar idle; the prologue warm-up
    (11 garbage matmuls reading the not-yet-DMA'd W_proj SBUF region)
    bridges queue-startup to the first qkv chains with zero idle.

Structure:
  - scores are computed TRANSPOSED, S^T[k,q] (lhsT=k^T, rhs=q^T), as 12
    passes (head pair p=i//2, q-half qh=i%2) of 4 BLOCKS of 2 kc-steps:
    [tiled: 2 score pairs + exps] then [untiled: norm_post / filler
    chains / projection chains / P@V of the previous pass, 2 steps'
    worth, evenly spread]. Fillers precede P@V so the pass-boundary
    normalize latency (DVE reciprocal -> GpSimd broadcast -> DVE muls)
    is covered before P@V reuses the freed accumulator slot.
  - softmax denominators come free via a ones-column in v (row 64 of
    the P@V accumulator); normalization multiplies write bf16 attn rows.
  - the qkv projection runs as 6-MM filler chains dispensed into the
    untiled segments, deadline-ordered; only q/k of head pair 0 runs in
    the prologue, interleaved by c-chunk with the x DMA landings (2 MMs
    per chunk keeps per-chunk waits ~0.2us, too short to re-throttle).
    DMA order: Wqkv[q0k0] -> x -> Wqkv[v] -> Wqkv[q1k1] -> rest -> Wproj.
  - projection y = attn @ W_proj^T + b_proj accumulates per 128-row
    chunk in staged PSUM sessions that hide inside the pass loop:
    A = c0-2 (+bias, added by DVE during PSUM->SBUF staging) dispensed
    into passes 7-9; B1 = c3-4 for n<4 in pass 10; single-MM c5 closers
    for n<4 in pass 11 blocks 2-3 (pass 10's P@V is front-loaded into
    blocks 0-1 so pair-5-qh0 rows normalize in-pass). For n>=4 the
    c3-c5 session runs in the epilogue on the freed score ring as
    [128,768] 2-bank tiles (c3/c4 fill the P@V(11)/normalize latency
    window; c5 + ONE [128,768] add + DMA close each row chunk).
"""

from collections import deque
from contextlib import ExitStack

import ml_dtypes
import numpy as np

import concourse.mybir as mybir
import concourse.tile as tile
from concourse import bacc
from concourse.bass_utils import run_bass_kernel_spmd

B, N, C = 8, 1024, 768
NH, D = 12, 64
CK = C // 128  # 6 contraction chunks of 128
NQ = N // 128  # 8 position chunks of 128
SCALE = D ** -0.5
F32 = mybir.dt.float32
BF16 = mybir.dt.bfloat16
Copy = mybir.ActivationFunctionType.Copy
Exp = mybir.ActivationFunctionType.Exp
BF = ml_dtypes.bfloat16


def _emit(tc, xT, wqkvT, wprojT, bproj, out, sim_safe=False):
    nc = tc.nc
    with ExitStack() as ctx:
        sb = ctx.enter_context(tc.tile_pool(name="sb", bufs=1))
        pp = ctx.enter_context(tc.tile_pool(name="pp", bufs=13))
        small = ctx.enter_context(tc.tile_pool(name="small", bufs=2))
        stage = ctx.enter_context(tc.tile_pool(name="stage", bufs=2))
        # PSUM budget (8 banks): score ring 2x[128,1024] (4 banks) + P@V
        # accumulator ring 3x[65,512] (3 banks) + filler/proj chain (1).
        acc = tc.alloc_tile_pool(name="acc", bufs=3, space="PSUM")
        ps = tc.alloc_tile_pool(name="ps", bufs=2, space="PSUM")
        fill = tc.alloc_tile_pool(name="fill", bufs=1, space="PSUM")
        warm_ps = ps.tile([128, 512], F32, name="warm_ps", tag="s")

        # ---- PE warm-up ----------------------------------------------
        # ~3us of matmuls so the HAM clock-gate opens (K=8/8) before the
        # first qk chains; the chains themselves keep it open.
        # ---- input DMAs (bf16, host-packed) --------------------------
        xT_bf = [
            sb.tile([128, N], BF16, name=f"xT_bf{c}", tag=f"xT_bf{c}")
            for c in range(CK)
        ]
        wq_bf = sb.tile([128, CK, 3 * C], BF16, name="wq_bf", tag="wq_bf")
        wp_bf = sb.tile([128, CK, C], BF16, name="wp_bf", tag="wp_bf")

        # warm-up matmuls read the NOT-YET-DMA'd W_proj region: garbage
        # values (results are discarded; the psum slot is overwritten by
        # the first score matmul's start=True), but zero gating — unlike
        # a memset, whose engine queue only starts ~4us in. The implied
        # read-before-write ordering delays only the W_proj DMA, which is
        # last in the queue and not needed until pass 7. CoreSim rejects
        # uninitialized reads, so sim_safe swaps in a memset-fed warmup
        # (numerically identical: the output is discarded either way).
        if sim_safe:
            warm_in = sb.tile([128, 512], BF16, name="warm_in", tag="warm_in")
            nc.gpsimd.memset(warm_in[:], 1.0)
            w_lhs, w_rhs = warm_in[:, 0:128], warm_in[:]
        else:
            w_lhs, w_rhs = wp_bf[:, 0, 0:128], wp_bf[:, 0, 0:512]
        # 11 matmuls ~= 4.7us at the cold clock: covers the HAM SHORT
        # window AND bridges to ~12us, where the first chain matmul can
        # actually issue (queue startup + DMA) — any PE idle gap here
        # would re-throttle the clock for the first chain MMs.
        for i in range(11):
            nc.tensor.matmul(
                warm_ps[:],
                lhsT=w_lhs,
                rhs=w_rhs,
                start=(i == 0),
                stop=(i == 10),
            )

        def dma_w(lo, hi):
            nc.sync.dma_start(
                out=wq_bf[:, :, lo:hi],
                in_=wqkvT[:, lo:hi].rearrange("(c p) w -> p c w", p=128),
            )

        dma_w(0, 256)  # q0|k0 -> prologue chains
        for c in range(CK):
            nc.sync.dma_start(
                out=xT_bf[c][:], in_=xT[c * 128:(c + 1) * 128, :]
            )
        dma_w(1536, 2304)  # v weights -> v filler chains (pass 0)
        dma_w(256, 512)  # q1|k1 -> qk(1)/qk(7) fillers (passes 0-1)
        dma_w(512, 1536)  # remaining q/k blocks
        nc.sync.dma_start(
            out=wp_bf[:], in_=wprojT.rearrange("(c p) w -> p c w", p=128)
        )
        bp_row = sb.tile([1, C], F32, name="bp_row", tag="bp_row")
        nc.sync.dma_start(out=bp_row[:], in_=bproj[None, :])

        # ---- qkv projections (filler chains) -------------------------
        # qkT[m] m 0..5 -> q rows of heads 2m,2m+1; 6..11 -> k rows.
        # Packed weight col offset: q_m at 256m, k_m at 256m+128.
        qkT = [
            sb.tile([128, N], BF16, name=f"qkT{m}", tag=f"qkT{m}")
            for m in range(12)
        ]

        def emit_qk_half(m, qh, pool=None, ptag="f"):
            co = 256 * m if m < 6 else 256 * (m - 6) + 128
            pool = pool or fill
            qk_ps = pool.tile([128, 512], F32, name=f"qk_ps{m}_{qh}", tag=ptag)
            for c in range(CK):
                nc.tensor.matmul(
                    qk_ps[:],
                    lhsT=wq_bf[:, c, co:co + 128],
                    rhs=xT_bf[c][:, qh * 512:(qh + 1) * 512],
                    start=(c == 0),
                    stop=(c == CK - 1),
                )
            nc.vector.tensor_copy(qkT[m][:, qh * 512:(qh + 1) * 512], qk_ps[:])

        v_sb = [
            sb.tile([128, NH, D + 1], BF16, name=f"v_sb{n}", tag=f"v_sb{n}")
            for n in range(NQ)
        ]

        def emit_v_half(n, half, pool=None, ptag="f"):
            if half == 0:
                nc.gpsimd.memset(v_sb[n][:, :, D], 1.0)
            pool = pool or fill
            v_ps = pool.tile([128, 384], F32, name=f"v_ps{n}_{half}", tag=ptag)
            for c in range(CK):
                nc.tensor.matmul(
                    v_ps[:],
                    lhsT=xT_bf[c][:, n * 128:(n + 1) * 128],
                    rhs=wq_bf[:, c, 1536 + half * 384:1536 + (half + 1) * 384],
                    start=(c == 0),
                    stop=(c == CK - 1),
                )
            nc.vector.tensor_copy(
                v_sb[n][:, half * 6:(half + 1) * 6, 0:D],
                v_ps[:].rearrange("p (h d) -> p h d", d=D),
            )

        # ---- prologue: q/k of head pair 0, DMA-pipelined -------------
        # The two chains interleave by c-chunk: 2 MMs per x chunk keeps
        # each per-chunk DMA wait short (~0.2us — far below the ~3.4us
        # idle window that would re-throttle the HAM clock gate, which a
        # sequential chain's ~1.5us waits do trip). Copies follow both.
        pA = ps.tile([128, 512], F32, name="pre0_0", tag="s")
        pB = ps.tile([128, 512], F32, name="pre6_0", tag="s")
        for c in range(CK):
            nc.tensor.matmul(
                pA[:], lhsT=wq_bf[:, c, 0:128], rhs=xT_bf[c][:, 0:512],
                start=(c == 0), stop=(c == CK - 1),
            )
            nc.tensor.matmul(
                pB[:], lhsT=wq_bf[:, c, 128:256], rhs=xT_bf[c][:, 0:512],
                start=(c == 0), stop=(c == CK - 1),
            )
        nc.vector.tensor_copy(qkT[0][:, 0:512], pA[:])
        nc.vector.tensor_copy(qkT[6][:, 0:512], pB[:])

        # ---- attention: 12 passes x 4 blocks of 2 steps --------------
        attn_bf = [
            sb.tile([128, N], BF16, name=f"attn_bf{p}", tag=f"attn_bf{p}")
            for p in range(6)
        ]

        def emit_S(i, kc):
            """Row-tiled score pair + exp for pass i=(p,qh), chunk kc."""
            p, qh = i // 2, i % 2
            q_tile, k_tile = qkT[p], qkT[6 + p]
            qs = slice(qh * 512, (qh + 1) * 512)
            st = ps.tile([128, N], F32, name=f"st{i}_{kc}", tag="s")
            nc.tensor.matmul(
                st[:, 0:512],
                lhsT=k_tile[0:D, kc * 128:(kc + 1) * 128],
                rhs=q_tile[0:D, qs],
                start=True,
                stop=True,
            )
            nc.tensor.matmul(
                st[:, 512:1024],
                lhsT=k_tile[D:128, kc * 128:(kc + 1) * 128],
                rhs=q_tile[D:128, qs],
                start=True,
                stop=True,
            )
            pt = pp.tile([128, N], BF16, name=f"P{i}_{kc}", tag="P")
            nc.scalar.activation(pt[:], st[:], Exp, scale=SCALE)
            return pt

        def emit_pv(i, oas, kc, pt):
            """P@V chunk kc for both heads of pass i=(p,qh)."""
            p = i // 2
            for half, oa in enumerate(oas):
                nc.tensor.matmul(
                    oa[:],
                    lhsT=v_sb[kc][:, 2 * p + half, :],
                    rhs=pt[:, half * 512:(half + 1) * 512],
                    start=(kc == 0),
                    stop=(kc == NQ - 1),
                )

        def emit_norm_pre(oas):
            """Reciprocal chain for the pass's two heads (DVE/GpSimd).
            Kept per-half: the two ~1us GpSimd broadcasts pipeline with
            other work, which beats one [64,1024] broadcast's latency."""
            bc = []
            for half in range(2):
                dn = small.tile([1, 512], F32, name=f"dn{half}", tag=f"dn{half}")
                nc.vector.tensor_copy(dn[:], oas[half][D:D + 1, :])
                rc = small.tile([1, 512], F32, name=f"rc{half}", tag=f"rc{half}")
                nc.vector.reciprocal_approx_fast(rc[:], dn[:])
                rcb = small.tile([1, 512], BF16, name=f"rcb{half}", tag=f"rcb{half}")
                nc.vector.tensor_copy(rcb[:], rc[:])
                b = small.tile([64, 512], BF16, name=f"bc{half}", tag=f"bc{half}")
                nc.gpsimd.partition_broadcast(b[:], rcb[:])
                bc.append(b)
            return bc

        def emit_norm_post(i, oas, bc):
            p, qh = i // 2, i % 2
            qs = slice(qh * 512, (qh + 1) * 512)
            for half in range(2):
                ro = half * 64
                nc.vector.tensor_mul(
                    attn_bf[p][ro:ro + 64, qs], oas[half][0:D, :], bc[half][:]
                )

        # ---- projection chains (A: c0-2+bias, B1: c3-4, C: c5) -------
        # y[n] = attn[n] @ Wp^T + b accumulates in three PSUM sessions so
        # nearly all of it hides inside the pass loop: A as soon as pairs
        # 0-2 are normalized (passes 8-9), B1 once pairs 3-4 are (passes
        # 10-11), and only the single-MM C closer (pair 5) + final add +
        # DMA remain in the epilogue.
        bias_bc = sb.tile([128, C], F32, name="bias_bc", tag="bias_bc")
        nc.gpsimd.partition_broadcast(bias_bc[:], bp_row[:])
        regions = [(0, 512), (512, 768)]
        projA_sb = [
            sb.tile([128, C], F32, name=f"pA{n}", tag=f"pA{n}") for n in range(NQ)
        ]
        projB_sb = [
            sb.tile([128, C], F32, name=f"pB{n}", tag=f"pB{n}") for n in range(NQ)
        ]

        def emit_projA(n, r):
            lo, hi = regions[r]
            a_ps = fill.tile([128, hi - lo], F32, name=f"aps{n}_{r}", tag="f")
            for c in range(3):
                nc.tensor.matmul(
                    a_ps[:],
                    lhsT=attn_bf[c][:, n * 128:(n + 1) * 128],
                    rhs=wp_bf[:, c, lo:hi],
                    start=(c == 0),
                    stop=(c == 2),
                )
            nc.vector.tensor_add(projA_sb[n][:, lo:hi], a_ps[:], bias_bc[:, lo:hi])

        def emit_projB1(n, r):
            lo, hi = regions[r]
            b_ps = fill.tile([128, hi - lo], F32, name=f"bps{n}_{r}", tag="f")
            for c in (3, 4):
                nc.tensor.matmul(
                    b_ps[:],
                    lhsT=attn_bf[c][:, n * 128:(n + 1) * 128],
                    rhs=wp_bf[:, c, lo:hi],
                    start=(c == 3),
                    stop=(c == 4),
                )
            nc.vector.tensor_add(
                projB_sb[n][:, lo:hi], b_ps[:], projA_sb[n][:, lo:hi]
            )

        def emit_projC_region(n, r):
            """In-pass closer (fill bank): c5 for one region + add."""
            lo, hi = regions[r]
            c_ps = fill.tile([128, hi - lo], F32, name=f"cps{n}_{r}", tag="f")
            nc.tensor.matmul(
                c_ps[:],
                lhsT=attn_bf[5][:, n * 128:(n + 1) * 128],
                rhs=wp_bf[:, 5, lo:hi],
                start=True,
                stop=True,
            )
            if r == 0:
                y_stage[n] = stage.tile([128, C], F32, name=f"y{n}", tag="y")
            nc.vector.tensor_add(y_stage[n][:, lo:hi], c_ps[:], projB_sb[n][:, lo:hi])
            if r == 1:
                nc.sync.dma_start(
                    out=out[n * 128:(n + 1) * 128, :], in_=y_stage[n][:]
                )

        def emit_projB_open(n, c, c_ps=None):
            """Epilogue n>=4 session (paused): one c's region MMs into a
            2-bank ps-pool tile; runs inside the PV(11)/norm window."""
            if c_ps is None:
                c_ps = ps.tile([128, C], F32, name=f"cps{n}", tag="s")
            for lo, hi in regions:
                nc.tensor.matmul(
                    c_ps[:, lo:hi],
                    lhsT=attn_bf[c][:, n * 128:(n + 1) * 128],
                    rhs=wp_bf[:, c, lo:hi],
                    start=(c == 3),
                    stop=False,
                )
            return c_ps

        def emit_projB_close(n, c_ps):
            """c5 region MMs (valid after norm(11)), ONE [128,768] add
            against the session-A partial, DMA."""
            for lo, hi in regions:
                nc.tensor.matmul(
                    c_ps[:, lo:hi],
                    lhsT=attn_bf[5][:, n * 128:(n + 1) * 128],
                    rhs=wp_bf[:, 5, lo:hi],
                    start=False,
                    stop=True,
                )
            y_stage[n] = stage.tile([128, C], F32, name=f"y{n}", tag="y")
            nc.vector.tensor_add(y_stage[n][:], c_ps[:], projA_sb[n][:])
            nc.sync.dma_start(out=out[n * 128:(n + 1) * 128, :], in_=y_stage[n][:])

        y_stage = [None] * NQ

        # ---- fill queues ---------------------------------------------
        # fills: deadline-ordered qkv chains. v(n,0) consumed by P@V of
        # pass 0 (during pass 1, block n//2); qk(m)/qk(6+m) by pass 2m;
        # v(n,1) first consumed by P@V of pass 6 (during pass 7).
        fills = deque()
        fills.append(lambda: emit_qk_half(6, 1))  # k pair0 pos 512:1024, pass 0 block 2
        fills.append(lambda: emit_qk_half(0, 1))  # q pair0 half 1, pass 1
        for n in range(NQ):
            fills.append(lambda n=n: emit_v_half(n, 0))
        for m in (7, 1):
            fills.append(lambda m=m: emit_qk_half(m, 0))
        for m in (7, 1):
            fills.append(lambda m=m: emit_qk_half(m, 1))
        for m in (8, 2):
            fills.append(lambda m=m: emit_qk_half(m, 0))
        for m in (8, 2):
            fills.append(lambda m=m: emit_qk_half(m, 1))
        for m in (9, 3):
            fills.append(lambda m=m: emit_qk_half(m, 0))
        for m in (9, 3):
            fills.append(lambda m=m: emit_qk_half(m, 1))
        for n in range(NQ):
            fills.append(lambda n=n: emit_v_half(n, 1))
        for m in (10, 4):
            fills.append(lambda m=m: emit_qk_half(m, 0))
        for m in (10, 4):
            fills.append(lambda m=m: emit_qk_half(m, 1))
        for m in (11, 5):
            fills.append(lambda m=m: emit_qk_half(m, 0))
        for m in (11, 5):
            fills.append(lambda m=m: emit_qk_half(m, 1))

        projA_q = deque((n, r) for n in range(NQ) for r in range(2))

        NPASS = 12
        prev = None  # (i, pts) of previous pass (P@V pending this pass)
        pend_post = None  # (i, oas, bc) awaiting norm_post
        for i in range(NPASS):
            if prev is not None:
                pi, ppts = prev
                poas = (
                    acc.tile([D + 1, 512], F32, name=f"oaA{pi}", tag="acc"),
                    acc.tile([D + 1, 512], F32, name=f"oaB{pi}", tag="acc"),
                )
            pts = {}
            for b in range(4):
                # tiled segment: this block's two score pairs + exps
                pts[2 * b] = emit_S(i, 2 * b)
                pts[2 * b + 1] = emit_S(i, 2 * b + 1)
                # untiled segment. norm_post first (frees the acc slot
                # this pass's P@V reuses), then fillers/proj chains —
                # independent PE work that covers the norm chain's
                # DVE/GpSimd latency — then the P@V matmuls.
                if b == 0 and pend_post is not None:
                    emit_norm_post(*pend_post)
                    pend_post = None
                nfill = 2 if i < 2 else 1
                for _ in range(nfill):
                    if fills:
                        fills.popleft()()
                if not fills and i <= 9 and projA_q:
                    emit_projA(*projA_q.popleft())
                    if projA_q:
                        emit_projA(*projA_q.popleft())
                # B1 sessions: pairs 3-4 rows are normalized by pass 10
                # block 0 (qh1 of pair 4 via norm_post(9) there), so n<4
                # chains run through pass 10 and n>=4 through pass 11.
                if i == 10:
                    emit_projB1(b, 0)
                    emit_projB1(b, 1)
                if prev is not None:
                    # pass 11 front-loads P@V(10) into blocks 0-1 so its
                    # normalize finishes in-pass and the pair-5-qh0 proj
                    # closers can run in blocks 2-3.
                    ks = (
                        (2 * b, 2 * b + 1) if i < 11
                        else (4 * b, 4 * b + 1, 4 * b + 2, 4 * b + 3) if b < 2
                        else ()
                    )
                    for k in ks:
                        emit_pv(pi, poas, k, ppts.pop(k))
                    if i == 11 and b == 1:
                        pend_post = (pi, poas, emit_norm_pre(poas))
                    if i == 11 and b == 2 and pend_post is not None:
                        emit_norm_post(*pend_post)
                        pend_post = None
                # pair-5-qh0 closers once norm_post(10) has landed
                if i == 11 and b >= 2:
                    for nn in ((0,) if b == 2 else (1, 2)):
                        emit_projC_region(nn, 0)
                        emit_projC_region(nn, 1)
            if i < 11 and prev is not None:
                pend_post = (pi, poas, emit_norm_pre(poas))
            prev = (i, pts)

        # ---- epilogue ------------------------------------------------
        # P@V + normalize of pass 11, C closers (c5) + final adds + DMA.
        li, lpts = prev
        loas = (
            acc.tile([D + 1, 512], F32, name=f"oaA{li}", tag="acc"),
            acc.tile([D + 1, 512], F32, name=f"oaB{li}", tag="acc"),
        )
        opens = {}
        for kc in range(NQ):
            emit_pv(li, loas, kc, lpts.pop(kc))
            # last n<4 closer (norm_post(10) landed in pass 11 block 2)
            if kc <= 1:
                emit_projC_region(3, kc)
            # n=4,5 c3/c4 sessions fill the PV/norm latency window
            elif kc == 2:
                opens[4] = emit_projB_open(4, 3)
            elif kc == 3:
                emit_projB_open(4, 4, opens[4])
            elif kc == 4:
                opens[5] = emit_projB_open(5, 3)
            elif kc == 5:
                emit_projB_open(5, 4, opens[5])
        fill.release()
        emit_norm_post(li, loas, emit_norm_pre(loas))
        # pair5 qh1 now normalized: c5 closers + one add + DMA per n
        emit_projB_close(4, opens[4])
        emit_projB_close(5, opens[5])
        for n in (6, 7):
            c_ps = emit_projB_open(n, 3)
            emit_projB_open(n, 4, c_ps)
            emit_projB_close(n, c_ps)
        ps.release()
        acc.release()


def build_graph(sim_safe=False):
    nc = bacc.Bacc("TRN2", target_bir_lowering=False, debug=False)
    xT = nc.declare_dram_parameter("xT", [C, N], BF16, isOutput=False)
    wqkvT = nc.declare_dram_parameter("wqkvT", [C, 3 * C], BF16, isOutput=False)
    wprojT = nc.declare_dram_parameter("wprojT", [C, C], BF16, isOutput=False)
    bproj = nc.declare_dram_parameter("bproj", [C], F32, isOutput=False)
    out = nc.declare_dram_parameter("out", [N, C], F32, isOutput=True)
    with tile.TileContext(nc) as tc:
        _emit(tc, xT.ap(), wqkvT.ap(), wprojT.ap(), bproj.ap(), out.ap(), sim_safe)
    nc.compile()
    return nc


_GRAPH = None


def _get_graph():
    global _GRAPH
    if _GRAPH is None:
        _GRAPH = build_graph()
    return _GRAPH


def make_in_maps(x, W_qkv, W_proj, b_proj):
    x = np.asarray(x, dtype=np.float32)
    wq = np.asarray(W_qkv, dtype=np.float32).T  # [C, 3C]; cols q|k|v
    # pack q/k column blocks interleaved: [q0|k0|q1|k1|...|q5|k5|v]
    packed = np.empty((C, 3 * C), dtype=BF)
    for m in range(6):
        packed[:, 256 * m:256 * m + 128] = wq[:, 128 * m:128 * (m + 1)]
        packed[:, 256 * m + 128:256 * (m + 1)] = wq[:, C + 128 * m:C + 128 * (m + 1)]
    packed[:, 1536:] = wq[:, 1536:]
    wprojT = np.ascontiguousarray(np.asarray(W_proj, dtype=np.float32).T.astype(BF))
    bp = np.ascontiguousarray(np.asarray(b_proj, dtype=np.float32))
    xT_all = np.ascontiguousarray(x.transpose(0, 2, 1).astype(BF))
    return [
        {"xT": xT_all[i], "wqkvT": packed, "wprojT": wprojT, "bproj": bp}
        for i in range(B)
    ]


def run(x, W_qkv, W_proj, b_proj, trace=False):
    nc = _get_graph()
    in_maps = make_in_maps(x, W_qkv, W_proj, b_proj)
    res = run_bass_kernel_spmd(nc, in_maps, core_ids=list(range(B)), trace=trace)
    out = np.stack([res.results[i]["out"] for i in range(B)], axis=0)
    return out.astype(np.float32, copy=False), res


def kernel(x, W_qkv, W_proj, b_proj, H=None, W=None):
    out, _ = run(x, W_qkv, W_proj, b_proj)
    return out
